# revision 38
# baseline (speedup 1.0000x reference)
"""Trainium2 Bass kernel for nn_DecoderBlockWithKeywords (fp8 version).

Decoder block: causal self-attn + gated (source-code / keywords) cross-attn
+ template cross-attn + FFN, with 4 LayerNorms.  B=4, T=1024, D=512, H=8,
dh=64, DFF=2048.

Sharding: pure data-parallel over (batch, query-half) -> 8 NeuronCores, no
collectives.  Each core computes 512 query tokens of one batch element.

v2 (fp8): all attention projections run in fp8e4 with DoubleRow perf mode
(2 contraction chunks per matmul, 2x PE throughput); AV matmuls pair kv
tiles via DoubleRow.  Weights are scaled x64 into fp8 (descale folded into
the psum eviction).  Scores stay at dh=64 contraction in fp8 (bf16-rate).
The FFN stays fully fp16 - it carries nearly all the quantization
sensitivity (measured: all-fp8 rel err 1.5e-2, fp16-FFN 4.4e-3).

v3: all inputs arrive host-pre-tiled ([P, NCH, cols], contiguous per
partition) and are loaded by a handful of large DMAs split across the two
HWDGE rings (sync + scalar), so PE starts ~4us in.  ALL K projections are
A8-scaled so every attention can split exp between DVE (Schraudolph: one
tensor_scalar add-B/max-0 written as int8 IS the fp8e4 bit pattern of
exp(logit-2); full tiles / even head-pairs) and ACT (causal-diagonal
tiles / odd head-pairs).  Softmax denominators come from a ones-column in
V; 1/n is computed directly from the PSUM row (reciprocal_approx_fast),
broadcast by a sel-matmul, and fused into a single (po*QS)*(1/n) STT per
head.  LN uses rstd = Exp(-0.5*Ln(var+eps)) - Ln+Exp share one ACT table
set.  Residual stream is fp16 end to end.
"""

import os
import sys
import math
import threading

import numpy as np

for _p in ("/opt/trn_rl_repo", "/root/.axon_site"):
    if os.path.isdir(_p) and _p not in sys.path:
        sys.path.append(_p)

import ml_dtypes
from contextlib import ExitStack

import concourse.bass as bass
import concourse.mybir as mybir
from concourse import bacc
from concourse.tile import TileContext

F16 = np.float16
F32 = np.float32
FP8 = ml_dtypes.float8_e4m3
NEG = -1000000.0
B, T, S, TM, KW, D, H, DFF = 4, 1024, 1024, 512, 64, 512, 8, 2048
KWP = 128  # keywords padded to one full kv tile
DH = D // H  # 64
P = 128
NCH = D // P  # 4 feature chunks
AF = mybir.ActivationFunctionType
OP = mybir.AluOpType
DR = mybir.MatmulPerfMode.DoubleRow

WS = 64.0                      # weight scale into fp8
QS = 4.0                       # Q/K/V-side eviction scale
A8 = 8.0 / math.log(2.0)       # Schraudolph slope for e4m3 bit pattern
B0 = 56.0 - 2.0 * A8           # Schraudolph bias (incl. exp bias -2)
VH = DH + 2                    # per-head V stride (64 vals + ones + pad)


# ---------------------------------------------------------------------------
# program builder
# ---------------------------------------------------------------------------

SAW_ORDER = ["sa_wk", "sa_wv", "sa_wq", "sa_wo"]
CCW_ORDER = ["cc_wk", "cc_wv", "ck_wk", "ck_wv",
             "cc_wq", "ck_wq", "cc_wo", "ck_wo"]
CTW_ORDER = ["ct_wk", "ct_wv", "ct_wq", "ct_wo"]


def build_program(qh, kts_cc, kts_ct, gate_b=0.0, apply_affine=False,
                  debug=False):
    f32, f16 = mybir.dt.float32, mybir.dt.float16
    fp8, i8 = mybir.dt.float8e4, mybir.dt.int8
    KV = 512 * (qh + 1)
    QOFF = qh * 512

    nc = bacc.Bacc("TRN2", target_bir_lowering=False, debug=False)

    def din(name, shape, dt=fp8):
        return nc.dram_tensor(name, shape, dt, kind="ExternalInput").ap()

    # All tensors arrive pre-tiled: [P, NCH, cols] with element (p, i, n) =
    # orig[i*128 + p, n], so every DMA is contiguous per partition.
    saw_d = din("saw", [P, NCH * 4 * D])
    ccw_d = din("ccw", [P, NCH * 8 * D])
    ctw_d = din("ctw", [P, NCH * 4 * D])
    xin_d = din("xin", [P, NCH * KV])
    xq16_d = din("xq16", [P, NCH * 512], f16)
    enc_cc_d = din("enc_cc", [P, NCH * (kts_cc + 1) * P])   # src | kw
    enc_ct_d = din("enc_ct", [P, NCH * kts_ct * P])         # tmpl
    w1d = din("ffn_w1", [P, NCH * DFF], f16)
    w2d = din("ffn_w2", [P, (DFF // P) * D])
    gw_d = din("gw", [P, 2 * NCH], f16)                     # gwA | gwB
    staird = din("stair", [P, P], f16)
    bias_d = din("biases", [P, 8], f32)  # ccA ccD ctA ctD kwA kwD pad pad
    affine_d = din("ln_affine", [P, NCH * 8], f32) if apply_affine else None
    outT = nc.dram_tensor("outT", [D, 512], f16, kind="ExternalOutput").ap()
    dbg_outs = {}

    def mkdbg(nm, shape):
        if nm not in dbg_outs:
            dbg_outs[nm] = nc.dram_tensor(f"dbg_{nm}", shape, f32,
                                          kind="ExternalOutput").ap()
        return dbg_outs[nm]

    with TileContext(nc, pool_alloc_mode="queue") as tc, ExitStack() as ctx:
        pers = ctx.enter_context(tc.tile_pool(name="pers", bufs=1))
        stair = pers.tile([P, P], f16, name="stair_t")
        nc.scalar.dma_start(out=stair, in_=staird)
        biases = pers.tile([P, 8], f32, name="biases_t")
        nc.scalar.dma_start(out=biases, in_=bias_d)
        gw_t = pers.tile([P, 2 * NCH], f16, name="gw_t")
        nc.scalar.dma_start(out=gw_t, in_=gw_d)
        ccbiasA = biases[:, 0:1]
        ccbiasD = biases[:, 1:2]
        ctbiasA = biases[:, 2:3]
        ctbiasD = biases[:, 3:4]
        kwbiasA = biases[:, 4:5]
        kwbiasD = biases[:, 5:6]
        gwa_t = gw_t[:, 0:NCH]
        gwb_t = gw_t[:, NCH:2 * NCH]
        ones_f = pers.tile([P, 1], f16, name="ones_f")
        nc.vector.memset(ones_f, 1.0)
        ones_row = pers.tile([1, P], f16, name="ones_row")
        nc.vector.memset(ones_row, 1.0)
        ones_row32 = pers.tile([1, P], f32, name="ones_row32")
        nc.vector.memset(ones_row32, 1.0)
        warm2 = pers.tile([P, 64], f16, name="warm2_t")
        nc.vector.memset(warm2, 0.0)
        neg2 = pers.tile([P, 1], f32, name="neg2_t")
        nc.vector.memset(neg2, -2.0)
        eps_t = pers.tile([1, 1], f32, name="eps_t")
        nc.vector.memset(eps_t, 1e-5)
        gb_t = pers.tile([1, 1], f32, name="gb_t")
        nc.vector.memset(gb_t, -float(gate_b))
        affine = None
        if apply_affine:
            affine = pers.tile([P, NCH * 8], f32, name="affine_t")
            nc.scalar.dma_start(out=affine, in_=affine_d)

        def tap(nm, tiles):
            if not debug:
                return
            cols = tiles[0].shape[-1]
            d = mkdbg(nm, [len(tiles) * P, cols])
            for i, t in enumerate(tiles):
                rows = t.shape[0]
                nc.gpsimd.dma_start(out=d[i * P:i * P + rows, :], in_=t)

        rpool = ctx.enter_context(tc.tile_pool(name="rpool", bufs=1))

        def mktiles(nm, cols=512, dt=f16, n=NCH, tagp=None):
            tagp = tagp or nm
            return [rpool.tile([P, cols], dt, name=f"{nm}{i}", tag=f"{tagp}{i}",
                               bufs=1) for i in range(n)]

        smallp = ctx.enter_context(tc.tile_pool(name="smallp", bufs=1))
        trp = ctx.enter_context(tc.tile_pool(name="trp", bufs=1))
        psA = ctx.enter_context(tc.tile_pool(name="psA", bufs=2, space="PSUM"))
        psB = ctx.enter_context(tc.tile_pool(name="psB", bufs=2, space="PSUM"))

        wt = {}

        def load_wgroup(pool, nm, dram_ap, names, q=None):
            """One contiguous DMA for a group of [P, NCH, D] weights."""
            t = pool.tile([P, NCH, len(names) * D], fp8, name=nm, tag=nm,
                          bufs=1)
            (q or nc.sync).dma_start(
                out=t, in_=dram_ap.rearrange("p (i n) -> p i n", i=NCH))
            for k, n in enumerate(names):
                wt[n] = t[:, :, k * D:(k + 1) * D]
            return t

        def load_act(pool, nm, dram_ap, cols, q=None):
            t = pool.tile([P, NCH, cols], fp8, name=nm, tag=nm, bufs=1)
            (q or nc.scalar).dma_start(
                out=t, in_=dram_ap.rearrange("p (i n) -> p i n", i=NCH))
            return t

        # ------------------------------------------------------------------
        # projections (fp8 DoubleRow over contraction pairs)
        # ------------------------------------------------------------------
        def proj_fm_gen(wn, rhs, ncols, evict, rhs_cs=0):
            """Weight-stationary: out feature-major.  rhs: [P, NCH, *] fp8."""
            ntt = (ncols + 511) // 512
            for j in range(NCH):
                for t in range(ntt):
                    cs = t * 512
                    ce = min(ncols, cs + 512)
                    ps = psA.tile([P, ce - cs], f32, name="proj_ps", tag="pps")
                    for g in range(2):
                        nc.tensor.matmul(
                            ps, wt[wn][:, 2 * g:2 * g + 2, j * P:(j + 1) * P],
                            rhs[:, 2 * g:2 * g + 2, rhs_cs + cs:rhs_cs + ce],
                            start=(g == 0), stop=(g == 1), perf_mode=DR)
                    evict(j, cs, ce, ps)
                    yield

        def drain(gen):
            for _ in gen:
                pass

        def keepwarm(n):
            """Dependency-free dummy matmuls (64-col, ~80ns each) to hold
            the HAM clock gate at K=8/8 through engine-serial chains (PE
            idle > ~3.4us drops the PE clock to 1.2 GHz and the re-warm
            costs another ~3.4us)."""
            dps = psA.tile([1, 64], f32, name="warm_ps", tag="pps")
            for _ in range(n):
                nc.tensor.matmul(dps, ones_f, warm2, start=True,
                                 stop=True, skip_group_check=True)

        def proj_fm(*a, **k):
            drain(proj_fm_gen(*a, **k))

        def proj_v_gen(enc, wn, nkv, vp_list, vpool, ktag):
            """V projection token-major into paired tiles [P,2,H*VH].
            All kv tiles are full 128 rows (host zero-pads)."""
            nkt = nkv // P
            for m in range(nkt):
                ps = psA.tile([P, D], f32, name="v_ps", tag="pps")
                for g in range(2):
                    nc.tensor.matmul(
                        ps, enc[:, 2 * g:2 * g + 2, m * P:(m + 1) * P],
                        wt[wn][:, 2 * g:2 * g + 2, :],
                        start=(g == 0), stop=(g == 1), perf_mode=DR)
                if m % 2 == 0:
                    vt = vpool.tile([P, 2, H * VH], fp8, name=f"{ktag}_v{m//2}",
                                    tag=f"{ktag}_v{m//2}", bufs=1)
                    vp_list.append(vt)
                vt = vp_list[m // 2]
                dst3 = vt[:, m % 2, :].rearrange("p (g c) -> p g c", c=VH)
                src3 = ps.rearrange("p (g c) -> p g c", c=DH)
                nc.vector.tensor_scalar_mul(dst3[:, :, 0:DH], src3, 1.0 / WS)
                nc.gpsimd.memset(dst3[:, :, DH:DH + 1], 1.0)
                yield
            if nkt % 2 == 1:  # odd tile count: zero the unused pair slot
                nc.gpsimd.memset(vp_list[-1][:, 1, :], 0.0)

        def proj_v(*a, **k):
            drain(proj_v_gen(*a, **k))

        # ------------------------------------------------------------------
        # attention
        # ------------------------------------------------------------------
        def attention_gen(qt, kt, vp_list, nkv, at, bias_a, bias_d, causal,
                          ktag, filler=None):
            """qt: [4][P,512] fp8 (2 heads per chunk); kt: [4][P,nkv] fp8
            (A8-scaled, scores psum = A8*logit); vp_list: paired V tiles;
            at: [P,NCH,512] fp8 out (normalized).  exp runs on DVE
            (Schraudolph int8 write) for full tiles / even head-pairs and
            on ACT for causal-diagonal tiles / odd head-pairs."""
            nkt = nkv // P
            npair = (nkt + 1) // 2
            for hp in (0, 2, 1, 3):
                po = [psB.tile([DH + 1, 512], f32, name=f"{ktag}_po{s}",
                               tag="x_po") for s in range(2)]
                for m in range(npair):
                    two = min(2, nkt - 2 * m)
                    pt = trp.tile([P, 2, 1024], fp8, name=f"{ktag}_pt",
                                  tag=f"pt_{ktag}",
                                  bufs={"sa": 3, "ck": 1}.get(ktag, 2))
                    for ti in range(two):
                        kt_i = 2 * m + ti
                        ps2 = psB.tile([P, 1024], f32, name=f"{ktag}_ps",
                                       tag="x_ps")
                        d = kt_i - (nkt - 4) if causal else -1
                        lo = d * P if (causal and d > 0) else 0
                        for s in range(2):
                            nc.tensor.matmul(
                                ps2[:, s * 512 + lo:(s + 1) * 512],
                                kt[hp][s * DH:(s + 1) * DH,
                                       kt_i * P:(kt_i + 1) * P],
                                qt[hp][s * DH:(s + 1) * DH, lo:512],
                                start=True, stop=True)
                        if causal and d >= 0:
                            for s in range(2):
                                o = s * 512
                                nc.vector.tensor_add(
                                    ps2[:, o + d * P:o + (d + 1) * P],
                                    ps2[:, o + d * P:o + (d + 1) * P], stair)
                                if d > 0:
                                    nc.gpsimd.memset(pt[:, ti, o:o + d * P], 0.0)
                                nc.scalar.activation(
                                    pt[:, ti, o + d * P:o + 512],
                                    ps2[:, o + d * P:o + 512],
                                    AF.Exp, bias=neg2[:, :],
                                    scale=1.0 / A8)
                        elif (kt_i + hp // 2) % 2 == 0:
                            bias = B0
                            if (not causal) and bias_d is not None \
                                    and kt_i == nkt - 1:
                                bias = bias_d
                            nc.vector.tensor_scalar(
                                pt[:, ti, :].bitcast(i8), ps2, bias, 0.0,
                                op0=OP.add, op1=OP.max)
                        else:
                            bias = neg2[:, :]
                            if bias_a is not None and kt_i == nkt - 1:
                                bias = bias_a
                            nc.scalar.activation(pt[:, ti, :], ps2, AF.Exp,
                                                 bias=bias, scale=1.0 / A8)
                        if two == 1:
                            nc.gpsimd.memset(pt[:, 1, :], 0.0)
                        if filler is not None:
                            if next(filler, False) is False:
                                keepwarm(4)
                        else:
                            keepwarm(4)
                    for s in range(2):
                        nc.tensor.matmul(
                            po[s],
                            vp_list[m][:, :, (2 * hp + s) * VH:
                                       (2 * hp + s) * VH + DH + 1],
                            pt[:, :, s * 512:(s + 1) * 512],
                            start=(m == 0), stop=(m == npair - 1),
                            perf_mode=DR)
                # denominator rows (ones-col in V): stage through SBUF
                # (custom-DVE bit tricks misread PSUM), recip on DVE, f16
                # cast on ACT, broadcast into nb's 64-row halves via two
                # small PE matmuls; numerators leave PSUM through ACT
                # copies (QS folded in), then one DVE mul applies 1/n
                nb = psA.tile([P, 512], f32, name=f"{ktag}_nb", tag="pps")
                at_tmp = trp.tile([P, 512], f16, name=f"{ktag}_att",
                                  tag="at_tmp", bufs=3)
                for s in range(2):
                    den_sb = smallp.tile([1, 512], f32,
                                         name=f"{ktag}_den{s}",
                                         tag="den_sb", bufs=4)
                    nc.scalar.activation(den_sb, po[s][DH:DH + 1, :],
                                         AF.Copy)
                    ninv = smallp.tile([1, 512], f32,
                                       name=f"{ktag}_ninv{s}",
                                       tag="ninv", bufs=4)
                    nc.vector.reciprocal_approx_fast(ninv, den_sb)
                    ninv16 = smallp.tile([1, 512], f16,
                                         name=f"{ktag}_ninv16{s}",
                                         tag="ninv16", bufs=4)
                    nc.scalar.activation(ninv16, ninv, AF.Copy)
                    nc.tensor.matmul(nb[s * DH:(s + 1) * DH, :],
                                     ones_row[:, 0:DH], ninv16,
                                     start=True, stop=True)
                    nc.scalar.activation(at_tmp[s * DH:(s + 1) * DH, :],
                                         po[s][0:DH, :], AF.Copy, scale=QS)
                nc.vector.tensor_mul(at[:, hp, :], at_tmp, nb)
                yield

        def attention(*a, **k):
            drain(attention_gen(*a, **k))

        # ------------------------------------------------------------------
        # layernorm (f16 residual in, f16 + fp8 out)
        # ------------------------------------------------------------------
        def layernorm(r_tiles, out16, out8, ln_idx):
            sq = [trp.tile([P, 512], f16, name=f"ln{ln_idx}_sq", tag="ln_sq",
                           bufs=2) for _ in range(NCH)]
            for j in range(NCH):
                if j < 2:
                    nc.scalar.activation(sq[j], r_tiles[j], AF.Square)
                elif j == 2:
                    nc.vector.tensor_mul(sq[j], r_tiles[j], r_tiles[j])
                else:
                    nc.gpsimd.tensor_mul(sq[j], r_tiles[j], r_tiles[j])
            ps_s = psB.tile([1, 512], f32, name="ln_ps_s", tag="x_po")
            ps_q = psB.tile([1, 512], f32, name="ln_ps_q", tag="x_po")
            for j in range(NCH):
                nc.tensor.matmul(ps_s, ones_f, r_tiles[j],
                                 start=(j == 0), stop=(j == NCH - 1))
            for j in range(NCH):
                nc.tensor.matmul(ps_q, ones_f, sq[j],
                                 start=(j == 0), stop=(j == NCH - 1))
            keepwarm(40)
            mean16 = smallp.tile([1, 512], f16, name="ln_mean16",
                                 tag="ln_stat", bufs=3)
            nc.scalar.activation(mean16, ps_s, AF.Copy, scale=1.0 / D)
            msq = smallp.tile([1, 512], f32, name="ln_msq",
                              tag="ln_stat", bufs=3)
            nc.scalar.activation(msq, ps_s, AF.Square, scale=1.0 / D)
            var = smallp.tile([1, 512], f32, name="ln_var",
                              tag="ln_stat", bufs=3)
            nc.vector.scalar_tensor_tensor(var, ps_q, 1.0 / D, msq,
                                           op0=OP.mult, op1=OP.subtract)
            # rstd via bitcast fast-rsqrt + one Newton step (no Ln table -
            # mixing Ln into the Exp-heavy kernel forces ~1.3us ACT table
            # swaps around every layernorm)
            lf = smallp.tile([1, 512], f32, name="ln_lf",
                             tag="ln_stat", bufs=3)
            nc.vector.tensor_scalar(lf, var.bitcast(mybir.dt.int32),
                                    1064866805.0, 8.2629582e-08,
                                    op0=OP.subtract, op1=OP.mult)
            rstd0 = smallp.tile([1, 512], f32, name="ln_rstd0",
                                tag="ln_nr", bufs=3)
            nc.scalar.activation(rstd0, lf, AF.Exp, scale=-0.5)
            t1 = smallp.tile([1, 512], f32, name="ln_t1", tag="ln_nr", bufs=3)
            nc.vector.tensor_mul(t1, rstd0, rstd0)
            nc.vector.tensor_mul(t1, t1, var)
            nc.vector.tensor_scalar(t1, t1, -0.5, 1.5, op0=OP.mult, op1=OP.add)
            rstd = smallp.tile([1, 512], f16, name="ln_rstd",
                               tag="ln_stat", bufs=3)
            nc.vector.tensor_mul(rstd, rstd0, t1)
            meanb = psB.tile([P, 512], f32, name="ln_meanb", tag="x_po")
            nc.tensor.matmul(meanb, ones_row, mean16, start=True, stop=True)
            rstdb = psB.tile([P, 512], f32, name="ln_rstdb", tag="x_po")
            nc.tensor.matmul(rstdb, ones_row, rstd, start=True, stop=True)
            keepwarm(15)
            mb_sb = trp.tile([P, 512], f16, name="ln_mb_sb", tag="ln_mb",
                             bufs=1)
            rb_sb = trp.tile([P, 512], f16, name="ln_rb_sb", tag="ln_rb",
                             bufs=1)
            nc.scalar.activation(mb_sb, meanb, AF.Copy)
            nc.vector.tensor_copy(rb_sb, rstdb)
            for j in range(NCH):
                tmp = trp.tile([P, 512], f16, name="ln_tmp", tag="ln_tmp",
                               bufs=2)
                if j >= 2:
                    nc.gpsimd.tensor_sub(tmp, r_tiles[j], mb_sb)
                    nc.gpsimd.tensor_mul(out16[j], tmp, rb_sb)
                else:
                    nc.vector.tensor_sub(tmp, r_tiles[j], mb_sb)
                    nc.vector.tensor_mul(out16[j], tmp, rb_sb)
                if apply_affine:
                    g = affine[:, ln_idx * 2 * NCH + j:
                               ln_idx * 2 * NCH + j + 1]
                    b = affine[:, ln_idx * 2 * NCH + NCH + j:
                               ln_idx * 2 * NCH + NCH + j + 1]
                    nc.vector.tensor_scalar(out16[j], out16[j], g, b,
                                            op0=OP.mult, op1=OP.add)
                if out8 is not None:
                    if j >= 2:
                        nc.gpsimd.tensor_copy(out8[:, j, :], out16[j])
                    else:
                        nc.vector.tensor_copy(out8[:, j, :], out16[j])

        # ==================================================================
        # emission
        # ==================================================================
        r1 = mktiles("r1", tagp="rA")
        y16 = mktiles("y", tagp="lnA")
        r2 = mktiles("r2", tagp="rB")
        z16 = mktiles("z", tagp="lnB")

        ctsb = ctx.enter_context(tc.tile_pool(name="tail_sb", bufs=1))
        ccsb_cm = tc.tile_pool(name="cc_sb", bufs=1)
        ccsb = ccsb_cm.__enter__()
        sasb_cm = tc.tile_pool(name="sa_sb", bufs=1)
        sasb = sasb_cm.__enter__()

        # --- stage 1: self attention ---
        # sync ring: saw -> xin -> xq16 -> w1 -> w2 (FIFO; sa starts ~4us)
        # scalar ring: consts (above) -> enc_cc -> ccw -> enc_ct -> ctw
        load_wgroup(sasb, "saw", saw_d, SAW_ORDER, q=nc.sync)
        xkv = load_act(sasb, "xkv", xin_d, KV, q=nc.sync)
        xq16_t = sasb.tile([P, NCH, 512], f16, name="xq16", tag="xq16",
                           bufs=1)
        nc.sync.dma_start(out=xq16_t,
                          in_=xq16_d.rearrange("p (i n) -> p i n", i=NCH))
        xq16 = [xq16_t[:, i, :] for i in range(NCH)]
        enc_cc_t = load_act(ccsb, "enc_cc", enc_cc_d, (kts_cc + 1) * P)
        srcl = enc_cc_t[:, :, 0:kts_cc * P]
        kwe = enc_cc_t[:, :, kts_cc * P:(kts_cc + 1) * P]
        load_wgroup(ccsb, "ccw", ccw_d, CCW_ORDER, q=nc.scalar)
        tmpl = load_act(ctsb, "enc_ct", enc_ct_d, kts_ct * P)
        load_wgroup(ctsb, "ctw", ctw_d, CTW_ORDER, q=nc.scalar)
        qt = [sasb.tile([P, 512], fp8, name=f"sa_q{i}", tag=f"sa_q{i}",
                        bufs=1) for i in range(NCH)]
        ktl = [sasb.tile([P, KV], fp8, name=f"sa_k{i}", tag=f"sa_k{i}",
                         bufs=1) for i in range(NCH)]
        keepwarm(110)

        def evict_scale(dst, scale):
            def ev(j, cs, ce, ps):
                if j % 2 == 0:
                    nc.vector.tensor_scalar_mul(dst[j][:, cs:ce], ps, scale)
                else:
                    nc.scalar.activation(dst[j][:, cs:ce], ps, AF.Copy,
                                         scale=scale)
            return ev

        proj_fm("sa_wk", xkv, KV, evict_scale(ktl, A8 / 2048.0))
        vts = []
        proj_v(xkv, "sa_wv", KV, vts, sasb, "sa")
        proj_fm("sa_wq", xkv, 512, evict_scale(qt, QS / WS), rhs_cs=QOFF)
        at = sasb.tile([P, NCH, 512], fp8, name="sa_at", tag="sa_at", bufs=1)
        # hoist: all cross-attn K/V projections as FILLER inside sa
        # attention so the PE queue stays dense through the exp stalls.
        cc_kt = [ccsb.tile([P, kts_cc * P], fp8, name=f"cc_k{i}",
                           tag=f"cc_k{i}", bufs=1) for i in range(NCH)]
        cc_vts = []
        ck_kt = [ccsb.tile([P, KWP], fp8, name=f"ck_k{i}", tag=f"ck_k{i}",
                           bufs=1) for i in range(NCH)]
        ck_vts = []
        ct_kt = [ctsb.tile([P, kts_ct * P], fp8, name=f"ct_k{i}",
                           tag=f"ct_k{i}", bufs=1) for i in range(NCH)]
        ct_vts = []

        def chain(*gens):
            for g in gens:
                yield from g

        # cc+ck K/V fill the sa-attention exp stalls; ct K/V is held back
        # so real PE + eviction work remains to fill the LN1 serial chain
        kv_filler = chain(
            proj_fm_gen("cc_wk", srcl, kts_cc * P,
                        evict_scale(cc_kt, A8 / 2048.0)),
            proj_v_gen(srcl, "cc_wv", kts_cc * P, cc_vts, ccsb, "cc"),
            proj_fm_gen("ck_wk", kwe, KWP, evict_scale(ck_kt, A8 / 2048.0)),
            proj_v_gen(kwe, "ck_wv", KWP, ck_vts, ccsb, "ck"),
        )
        late_kv = chain(
            proj_fm_gen("ct_wk", tmpl, kts_ct * P,
                        evict_scale(ct_kt, A8 / 2048.0)),
            proj_v_gen(tmpl, "ct_wv", kts_ct * P, ct_vts, ctsb, "ct"),
        )
        attention(qt, ktl, vts, KV, at, None, None, True, "sa",
                  filler=kv_filler)

        def evict_resid(dst, src16, scale):
            def ev(j, cs, ce, ps):
                nc.vector.scalar_tensor_tensor(
                    dst[j][:, cs:ce], ps, scale, src16[j][:, cs:ce],
                    op0=OP.mult, op1=OP.add)
            return ev

        keepwarm(20)
        proj_fm("sa_wo", at, 512, evict_resid(r1, xq16, 1.0 / (QS * WS)))
        tap("r1", r1)
        y8 = ccsb.tile([P, NCH, 512], fp8, name="y8", tag="y8", bufs=1)
        layernorm(r1, y16, y8, 0)
        drain(kv_filler)
        drain(late_kv)
        tap("y", y16)
        sasb_cm.__exit__(None, None, None)

        # --- stage 2: cc + ck cross attention + gate ---
        cc_qt = [ccsb.tile([P, 512], fp8, name=f"cc_q{i}", tag=f"cc_q{i}",
                           bufs=1) for i in range(NCH)]
        proj_fm("cc_wq", y8, 512, evict_scale(cc_qt, QS / WS))
        cc_at = ccsb.tile([P, NCH, 512], fp8, name="cc_at", tag="cc_at",
                          bufs=1)
        ck_qt = [ccsb.tile([P, 512], fp8, name=f"ck_q{i}", tag=f"ck_q{i}",
                           bufs=1) for i in range(NCH)]
        cc_filler = proj_fm_gen("ck_wq", y8, 512, evict_scale(ck_qt, QS / WS))
        ck_at = ccsb.tile([P, NCH, 512], fp8, name="ck_at", tag="ck_at",
                          bufs=1)
        attention(cc_qt, cc_kt, cc_vts, kts_cc * P, cc_at,
                  ccbiasA, ccbiasD, False, "cc", filler=cc_filler)
        drain(cc_filler)
        attention(ck_qt, ck_kt, ck_vts, KWP, ck_at, kwbiasA, kwbiasD,
                  False, "ck")
        y2c = mktiles("y2c", tagp="y2c")
        y2k = mktiles("y2k", tagp="y2k")
        keepwarm(20)
        proj_fm("cc_wo", cc_at, 512, evict_scale(y2c, 1.0 / (QS * WS)))
        proj_fm("ck_wo", ck_at, 512, evict_scale(y2k, 1.0 / (QS * WS)))

        # --- gate ---
        ps_g = psB.tile([1, 512], f32, name="gate_ps", tag="x_po")
        for i in range(NCH):
            nc.tensor.matmul(ps_g, gwa_t[:, i:i + 1], y2c[i],
                             start=(i == 0), stop=False)
        for i in range(NCH):
            nc.tensor.matmul(ps_g, gwb_t[:, i:i + 1], y2k[i],
                             start=False, stop=(i == NCH - 1))
        ge = smallp.tile([1, 512], f32, name="gate_e", tag="gate_e", bufs=2)
        nc.scalar.activation(ge, ps_g, AF.Exp, scale=-1.0, bias=gb_t[:, :])
        ge1 = smallp.tile([1, 512], f32, name="gate_e1", tag="gate_e", bufs=2)
        nc.vector.tensor_scalar_add(ge1, ge, 1.0)
        gg = smallp.tile([1, 512], f32, name="gate_g", tag="gate_g", bufs=2)
        nc.vector.reciprocal_approx_fast(gg, ge1)
        gg16 = smallp.tile([1, 512], f16, name="gate_g16", tag="gate_g16",
                           bufs=1)
        nc.vector.tensor_copy(gg16, gg)
        keepwarm(35)
        ggb = psB.tile([P, 512], f32, name="gate_gb", tag="x_po")
        nc.tensor.matmul(ggb, ones_row, gg16, start=True, stop=True)
        ggb_sb = trp.tile([P, 512], f16, name="ggb_sb", tag="ggb_sb", bufs=1)
        nc.scalar.activation(ggb_sb, ggb, AF.Copy)
        # r2 = y + y2k + g*(y2c - y2k)   (blends split DVE / gpsimd)
        for j in range(NCH):
            dt_ = trp.tile([P, 512], f16, name="gate_dt", tag="gate_dt",
                           bufs=4)
            if j >= 2:
                nc.gpsimd.tensor_sub(dt_, y2c[j], y2k[j])
                nc.gpsimd.tensor_mul(dt_, dt_, ggb_sb)
                nc.gpsimd.tensor_add(r2[j], y16[j], y2k[j])
                nc.gpsimd.tensor_add(r2[j], r2[j], dt_)
            else:
                nc.vector.tensor_sub(dt_, y2c[j], y2k[j])
                nc.vector.tensor_mul(dt_, dt_, ggb_sb)
                nc.vector.tensor_add(r2[j], y16[j], y2k[j])
                nc.vector.tensor_add(r2[j], r2[j], dt_)
        tap("r2", r2)
        ccsb_cm.__exit__(None, None, None)
        ffsb = ctx.enter_context(tc.tile_pool(name="ff_sb", bufs=1))
        w1t = ffsb.tile([P, NCH, DFF], f16, name="w1_t", tag="w1_t")
        nc.sync.dma_start(out=w1t,
                          in_=w1d.rearrange("p (i n) -> p i n", i=NCH))
        w2t = ffsb.tile([P, DFF // P, D], fp8, name="w2_t", tag="w2_t")
        nc.sync.dma_start(out=w2t,
                          in_=w2d.rearrange("p (i n) -> p i n", i=DFF // P))
        z8 = ctsb.tile([P, NCH, 512], fp8, name="z8", tag="z8", bufs=1)
        layernorm(r2, z16, z8, 1)
        tap("z", z16)

        # --- stage 3: ct cross attention ---
        r3 = mktiles("r3", tagp="rA")
        ze16 = mktiles("ze", tagp="lnA")
        ct_qt = [ctsb.tile([P, 512], fp8, name=f"ct_q{i}", tag=f"ct_q{i}",
                           bufs=1) for i in range(NCH)]
        proj_fm("ct_wq", z8, 512, evict_scale(ct_qt, QS / WS))
        ct_at = ctsb.tile([P, NCH, 512], fp8, name="ct_at", tag="ct_at",
                          bufs=1)
        attention(ct_qt, ct_kt, ct_vts, kts_ct * P, ct_at,
                  ctbiasA, ctbiasD, False, "ct")
        keepwarm(20)
        proj_fm("ct_wo", ct_at, 512, evict_resid(r3, z16, 1.0 / (QS * WS)))
        tap("r3", r3)
        layernorm(r3, ze16, None, 2)
        tap("ze", ze16)

        # --- stage 4: FFN ---
        ht = ffsb.tile([P, DFF // P, 512], fp8, name="ff_h", tag="ff_h",
                       bufs=1)
        for jf in range(DFF // P):
            ps = psA.tile([P, 512], f32, name="ff_ps", tag="pps")
            for i in range(NCH):
                nc.tensor.matmul(ps, w1t[:, i, jf * P:(jf + 1) * P],
                                 ze16[i], start=(i == 0), stop=(i == NCH - 1))
            if jf % 2 == 0:
                nc.scalar.activation(ht[:, jf, :], ps, AF.Relu, scale=QS)
            else:
                nc.vector.tensor_scalar(ht[:, jf, :], ps, QS, 0.0,
                                        op0=OP.mult, op1=OP.max)
        r4 = mktiles("r4", tagp="rB")
        for j in range(NCH):
            ps = psA.tile([P, 512], f32, name="ff_ps2", tag="pps")
            for g in range(DFF // P // 2):
                nc.tensor.matmul(ps, w2t[:, 2 * g:2 * g + 2, j * P:(j + 1) * P],
                                 ht[:, 2 * g:2 * g + 2, :], start=(g == 0),
                                 stop=(g == DFF // P // 2 - 1), perf_mode=DR)
            nc.vector.scalar_tensor_tensor(r4[j], ps, 1.0 / (QS * WS),
                                           ze16[j], op0=OP.mult, op1=OP.add)
        fin = [trp.tile([P, 512], f16, name=f"fin{i}", tag=f"fin{i}",
                        bufs=1) for i in range(NCH)]
        layernorm(r4, fin, None, 3)
        outq = [nc.sync, nc.scalar, nc.sync, nc.scalar]
        for j in range(NCH):
            outq[j].dma_start(out=outT[j * P:(j + 1) * P, :], in_=fin[j])

    nc.compile()
    return nc


# ---------------------------------------------------------------------------
# host-side input preparation
# ---------------------------------------------------------------------------

def _fp8(x, scale=1.0):
    return np.ascontiguousarray(
        (np.asarray(x, F32) * scale).astype(FP8))


def _tile_feat(a):
    """[D, N] -> [P, NCH, N]; element (p, i, n) = a[i*128 + p, n]."""
    a = np.asarray(a)
    return a.reshape(NCH, P, a.shape[1]).transpose(1, 0, 2)


def _prep_shared(inputs):
    sh = {}
    w8 = {f"{n}_{p}": _fp8(inputs[f"{n}_{p}"], WS)
          for n in ("sa", "cc", "ct", "ck") for p in ("wq", "wk", "wv", "wo")}

    def pack(names):
        return np.ascontiguousarray(np.concatenate(
            [_tile_feat(w8[n]) for n in names], axis=2)).reshape(P, -1)

    sh["saw"] = pack(SAW_ORDER)
    sh["ccw"] = pack(CCW_ORDER)
    sh["ctw"] = pack(CTW_ORDER)
    sh["ffn_w1"] = np.ascontiguousarray(
        _tile_feat(np.asarray(inputs["ffn_w1"], F32).astype(F16))
    ).reshape(P, -1)
    w2 = _fp8(inputs["ffn_w2"], WS)   # [DFF, D]
    sh["ffn_w2"] = np.ascontiguousarray(
        w2.reshape(DFF // P, P, D).transpose(1, 0, 2)).reshape(P, -1)
    gw = np.asarray(inputs["gate_w"], F32)
    sh["gw"] = np.ascontiguousarray(np.concatenate(
        [gw[:D, 0].reshape(NCH, P).T, gw[D:, 0].reshape(NCH, P).T],
        axis=1).astype(F16))
    kl, ql = np.arange(P)[:, None], np.arange(P)[None, :]
    sh["stair"] = np.where(kl <= ql, 0.0, NEG).astype(F16)
    return sh


def _len_bias(L, kts, base_val, width=P):
    base = (kts - 1) * P
    idx = base + np.arange(width)
    return (base_val + np.where(idx < L, 0.0, NEG)).astype(F32)[:, None]


def _prep_core(inputs, sh, b, qh, kts_cc, kts_ct):
    KVn = 512 * (qh + 1)
    m = dict(sh)
    xT = np.asarray(inputs["x"][b], F32).T  # [D, T]
    m["xin"] = np.ascontiguousarray(
        _tile_feat(_fp8(xT[:, :KVn]))).reshape(P, -1)
    m["xq16"] = np.ascontiguousarray(
        _tile_feat(xT[:, qh * 512:(qh + 1) * 512].astype(F16))).reshape(P, -1)
    Ls = int(inputs["source_code_len"][b])
    st = np.zeros((D, kts_cc * P), FP8)
    st[:, :Ls] = _fp8(np.asarray(inputs["source_code_enc"][b], F32)[:Ls].T)
    kwp = np.zeros((D, KWP), FP8)
    kwp[:, :KW] = _fp8(np.asarray(inputs["keywords_enc"][b], F32).T)
    m["enc_cc"] = np.ascontiguousarray(np.concatenate(
        [_tile_feat(st), _tile_feat(kwp)], axis=2)).reshape(P, -1)
    Lt = int(inputs["template_len"][b])
    tt = np.zeros((D, kts_ct * P), FP8)
    tt[:, :Lt] = _fp8(np.asarray(inputs["template_enc"][b], F32)[:Lt].T)
    m["enc_ct"] = np.ascontiguousarray(_tile_feat(tt)).reshape(P, -1)
    Lk = int(inputs["keywords_len"][b])
    bz = np.zeros((P, 8), F32)
    bz[:, 0:1] = _len_bias(Ls, kts_cc, -2.0)
    bz[:, 1:2] = _len_bias(Ls, kts_cc, B0)
    bz[:, 2:3] = _len_bias(Lt, kts_ct, -2.0)
    bz[:, 3:4] = _len_bias(Lt, kts_ct, B0)
    bz[:, 4:5] = _len_bias(Lk, 1, -2.0, P)
    bz[:, 5:6] = _len_bias(Lk, 1, B0, P)
    m["biases"] = bz
    return m


# ---------------------------------------------------------------------------
# concurrent multi-program PJRT runner
# ---------------------------------------------------------------------------

def _run_groups(groups):
    import jax
    import numpy as _np
    from jax.sharding import Mesh, PartitionSpec
    from jax.experimental.shard_map import shard_map
    from concourse import bass2jax
    from concourse.bass2jax import (_bass_exec_p, install_neuronx_cc_hook,
                                    partition_id_tensor)

    install_neuronx_cc_hook()
    devices = jax.devices()

    def make_launch(nc, core_ids, in_maps):
        pname = (nc.partition_id_tensor.name
                 if nc.partition_id_tensor else None)
        in_names, out_names, out_avals, zero_outs = [], [], [], []
        for alloc in nc.m.functions[0].allocations:
            if not isinstance(alloc, mybir.MemoryLocationSet):
                continue
            name = alloc.memorylocations[0].name
            if alloc.kind == "ExternalInput":
                if name == pname:
                    continue
                in_names.append(name)
            elif alloc.kind == "ExternalOutput":
                shape = tuple(alloc.tensor_shape)
                dtype = mybir.dt.np(alloc.dtype)
                out_names.append(name)
                out_avals.append(jax.core.ShapedArray(shape, dtype))
                zero_outs.append(_np.zeros(shape, dtype))
        n_params, n_outs = len(in_names), len(out_avals)
        all_in_names = in_names + out_names
        if pname is not None:
            all_in_names = all_in_names + [pname]

        def _body(*args):
            operands = list(args)
            if pname is not None:
                operands.append(partition_id_tensor())
            outs = _bass_exec_p.bind(
                *operands, out_avals=tuple(out_avals),
                in_names=tuple(all_in_names), out_names=tuple(out_names),
                lowering_input_output_aliases=(),
                sim_require_finite=False, sim_require_nnan=False, nc=nc)
            return tuple(outs)

        donate = tuple(range(n_params, n_params + n_outs))
        devs = [devices[c] for c in core_ids]
        if len(core_ids) == 1:
            fn = jax.jit(_body, donate_argnums=donate, keep_unused=True,
                         device=devs[0])
            args = [in_maps[0][nm] for nm in in_names] + list(zero_outs)
            out_arrs = fn(*args)
            return out_names, out_avals, out_arrs, None
        mesh = Mesh(_np.asarray(devs), ("core",))
        in_specs = (PartitionSpec("core"),) * (n_params + n_outs)
        out_specs = (PartitionSpec("core"),) * n_outs
        fn = jax.jit(shard_map(_body, mesh=mesh, in_specs=in_specs,
                               out_specs=out_specs, check_rep=False),
                     donate_argnums=donate, keep_unused=True)
        cat = [_np.concatenate([_np.asarray(m[nm]) for m in in_maps], axis=0)
               for nm in in_names]
        catz = [_np.zeros((len(core_ids) * z.shape[0], *z.shape[1:]), z.dtype)
                for z in zero_outs]
        out_arrs = fn(*cat, *catz)
        return out_names, out_avals, out_arrs, len(core_ids)

    last_err = None
    for _attempt in range(3):
        try:
            launched = []
            for nc, core_ids, in_maps in groups:
                launched.append((core_ids, make_launch(nc, core_ids, in_maps)))
            results = {}
            for core_ids, (out_names, out_avals, out_arrs, ncores) in launched:
                if ncores is None:
                    results[core_ids[0]] = {nm: _np.asarray(out_arrs[i])
                                            for i, nm in enumerate(out_names)}
                else:
                    for ci, c in enumerate(core_ids):
                        results[c] = {
                            nm: _np.asarray(out_arrs[i]).reshape(
                                ncores, *out_avals[i].shape)[ci]
                            for i, nm in enumerate(out_names)}
            return results
        except Exception as e:
            last_err = e
            import time as _time
            _time.sleep(2.0)
    raise last_err


_PROGRAM_CACHE = {}
_CACHE_LOCK = threading.Lock()


def _get_program(key):
    with _CACHE_LOCK:
        if key in _PROGRAM_CACHE:
            return _PROGRAM_CACHE[key]
    qh, kts_cc, kts_ct, gate_b, aff = key
    nc = build_program(qh, kts_cc, kts_ct, gate_b=gate_b, apply_affine=aff)
    with _CACHE_LOCK:
        _PROGRAM_CACHE[key] = nc
    return nc


# ---------------------------------------------------------------------------
# entry point
# ---------------------------------------------------------------------------

def kernel(**inputs):
    inputs = {k: np.asarray(v) for k, v in inputs.items()}
    gate_b = float(inputs["gate_b"].reshape(-1)[0])
    aff = not all(
        np.all(inputs[f"ln{j}_g"] == 1.0) and np.all(inputs[f"ln{j}_b"] == 0.0)
        for j in range(1, 5))
    affine_arr = None
    if aff:
        affine_arr = np.zeros((P, NCH * 8), F32)
        for ln in range(4):
            g = inputs[f"ln{ln + 1}_g"].astype(F32).reshape(NCH, P).T
            bb = inputs[f"ln{ln + 1}_b"].astype(F32).reshape(NCH, P).T
            affine_arr[:, ln * 2 * NCH: ln * 2 * NCH + NCH] = g
            affine_arr[:, ln * 2 * NCH + NCH: (ln + 1) * 2 * NCH] = bb

    sh = _prep_shared(inputs)
    core_keys, core_maps = [], []
    for c in range(8):
        b, qh = c // 2, c % 2
        kts_cc = max(1, -(-int(inputs["source_code_len"][b]) // P))
        kts_ct = max(1, -(-int(inputs["template_len"][b]) // P))
        key = (qh, kts_cc, kts_ct, gate_b, aff)
        m = _prep_core(inputs, sh, b, qh, kts_cc, kts_ct)
        if aff:
            m["ln_affine"] = affine_arr
        core_keys.append(key)
        core_maps.append(m)

    distinct = sorted(set(core_keys))
    threads = [threading.Thread(target=_get_program, args=(k,))
               for k in distinct]
    for t in threads:
        t.start()
    for t in threads:
        t.join()

    groups = []
    for key in distinct:
        cores = [c for c in range(8) if core_keys[c] == key]
        groups.append((_get_program(key), cores, [core_maps[c] for c in cores]))

    results = _run_groups(groups)

    out = np.empty((B, T, D), np.float32)
    for c in range(8):
        b, qh = c // 2, c % 2
        out[b, qh * 512:(qh + 1) * 512, :] = \
            results[c]["outT"].astype(np.float32).T
    return out



# revision 43
# speedup vs baseline: 1.0002x; 1.0002x over previous
"""Trainium2 Bass kernel for nn_DecoderBlockWithKeywords (fp8 version).

Decoder block: causal self-attn + gated (source-code / keywords) cross-attn
+ template cross-attn + FFN, with 4 LayerNorms.  B=4, T=1024, D=512, H=8,
dh=64, DFF=2048.

Sharding: pure data-parallel over (batch, query-half) -> 8 NeuronCores, no
collectives.  Each core computes 512 query tokens of one batch element.

v2 (fp8): all attention projections run in fp8e4 with DoubleRow perf mode
(2 contraction chunks per matmul, 2x PE throughput); AV matmuls pair kv
tiles via DoubleRow.  Weights are scaled x64 into fp8 (descale folded into
the psum eviction).  Scores stay at dh=64 contraction in fp8 (bf16-rate).
The FFN stays fully fp16 - it carries nearly all the quantization
sensitivity (measured: all-fp8 rel err 1.5e-2, fp16-FFN 4.4e-3).

v3: all inputs arrive host-pre-tiled ([P, NCH, cols], contiguous per
partition) and are loaded by a handful of large DMAs split across the two
HWDGE rings (sync + scalar), so PE starts ~4us in.  ALL K projections are
A8-scaled so every attention can split exp between DVE (Schraudolph: one
tensor_scalar add-B/max-0 written as int8 IS the fp8e4 bit pattern of
exp(logit-2); full tiles / even head-pairs) and ACT (causal-diagonal
tiles / odd head-pairs).  Softmax denominators come from a ones-column in
V; 1/n is computed directly from the PSUM row (reciprocal_approx_fast),
broadcast by a sel-matmul, and fused into a single (po*QS)*(1/n) STT per
head.  LN uses rstd = Exp(-0.5*Ln(var+eps)) - Ln+Exp share one ACT table
set.  Residual stream is fp16 end to end.
"""

import os
import sys
import math
import threading

import numpy as np

for _p in ("/opt/trn_rl_repo", "/root/.axon_site"):
    if os.path.isdir(_p) and _p not in sys.path:
        sys.path.append(_p)

import ml_dtypes
from contextlib import ExitStack

import concourse.bass as bass
import concourse.mybir as mybir
from concourse import bacc
from concourse.tile import TileContext

F16 = np.float16
F32 = np.float32
FP8 = ml_dtypes.float8_e4m3
NEG = -1000000.0
B, T, S, TM, KW, D, H, DFF = 4, 1024, 1024, 512, 64, 512, 8, 2048
KWP = 128  # keywords padded to one full kv tile
DH = D // H  # 64
P = 128
NCH = D // P  # 4 feature chunks
AF = mybir.ActivationFunctionType
OP = mybir.AluOpType
DR = mybir.MatmulPerfMode.DoubleRow

WS = 64.0                      # weight scale into fp8
QS = 4.0                       # Q/K/V-side eviction scale
A8 = 8.0 / math.log(2.0)       # Schraudolph slope for e4m3 bit pattern
B0 = 56.0 - 2.0 * A8           # Schraudolph bias (incl. exp bias -2)
VH = DH + 2                    # per-head V stride (64 vals + ones + pad)


# ---------------------------------------------------------------------------
# program builder
# ---------------------------------------------------------------------------

SAW_ORDER = ["sa_wk", "sa_wv", "sa_wq", "sa_wo"]
CCW_ORDER = ["cc_wk", "cc_wv", "ck_wk", "ck_wv",
             "cc_wq", "ck_wq", "cc_wo", "ck_wo"]
CTW_ORDER = ["ct_wk", "ct_wv", "ct_wq", "ct_wo"]


def build_program(qh, kts_cc, kts_ct, gate_b=0.0, apply_affine=False,
                  debug=False):
    f32, f16 = mybir.dt.float32, mybir.dt.float16
    fp8, i8 = mybir.dt.float8e4, mybir.dt.int8
    KV = 512 * (qh + 1)
    QOFF = qh * 512

    nc = bacc.Bacc("TRN2", target_bir_lowering=False, debug=False)

    def din(name, shape, dt=fp8):
        return nc.dram_tensor(name, shape, dt, kind="ExternalInput").ap()

    # All tensors arrive pre-tiled: [P, NCH, cols] with element (p, i, n) =
    # orig[i*128 + p, n], so every DMA is contiguous per partition.
    saw_d = din("saw", [P, NCH * 4 * D])
    ccw_d = din("ccw", [P, NCH * 8 * D])
    ctw_d = din("ctw", [P, NCH * 4 * D])
    xin_d = din("xin", [P, NCH * KV])
    xq16_d = din("xq16", [P, NCH * 512], f16)
    enc_cc_d = din("enc_cc", [P, NCH * (kts_cc + 1) * P])   # src | kw
    enc_ct_d = din("enc_ct", [P, NCH * kts_ct * P])         # tmpl
    w1d = din("ffn_w1", [P, NCH * DFF], f16)
    w2d = din("ffn_w2", [P, (DFF // P) * D])
    gw_d = din("gw", [P, 2 * NCH], f16)                     # gwA | gwB
    staird = din("stair", [P, P], f16)
    bias_d = din("biases", [P, 8], f32)  # ccA ccD ctA ctD kwA kwD pad pad
    affine_d = din("ln_affine", [P, NCH * 8], f32) if apply_affine else None
    outT = nc.dram_tensor("outT", [D, 512], f16, kind="ExternalOutput").ap()
    dbg_outs = {}

    def mkdbg(nm, shape):
        if nm not in dbg_outs:
            dbg_outs[nm] = nc.dram_tensor(f"dbg_{nm}", shape, f32,
                                          kind="ExternalOutput").ap()
        return dbg_outs[nm]

    with TileContext(nc, pool_alloc_mode="queue") as tc, ExitStack() as ctx:
        pers = ctx.enter_context(tc.tile_pool(name="pers", bufs=1))
        stair = pers.tile([P, P], f16, name="stair_t")
        nc.scalar.dma_start(out=stair, in_=staird)
        biases = pers.tile([P, 8], f32, name="biases_t")
        nc.scalar.dma_start(out=biases, in_=bias_d)
        gw_t = pers.tile([P, 2 * NCH], f16, name="gw_t")
        nc.scalar.dma_start(out=gw_t, in_=gw_d)
        ccbiasA = biases[:, 0:1]
        ccbiasD = biases[:, 1:2]
        ctbiasA = biases[:, 2:3]
        ctbiasD = biases[:, 3:4]
        kwbiasA = biases[:, 4:5]
        kwbiasD = biases[:, 5:6]
        gwa_t = gw_t[:, 0:NCH]
        gwb_t = gw_t[:, NCH:2 * NCH]
        ones_f = pers.tile([P, 1], f16, name="ones_f")
        nc.vector.memset(ones_f, 1.0)
        ones_row = pers.tile([1, P], f16, name="ones_row")
        nc.vector.memset(ones_row, 1.0)
        ones_row32 = pers.tile([1, P], f32, name="ones_row32")
        nc.vector.memset(ones_row32, 1.0)
        warm2 = pers.tile([P, 64], f16, name="warm2_t")
        nc.vector.memset(warm2, 0.0)
        neg2 = pers.tile([P, 1], f32, name="neg2_t")
        nc.vector.memset(neg2, -2.0)
        eps_t = pers.tile([1, 1], f32, name="eps_t")
        nc.vector.memset(eps_t, 1e-5)
        gb_t = pers.tile([1, 1], f32, name="gb_t")
        nc.vector.memset(gb_t, -float(gate_b))
        affine = None
        if apply_affine:
            affine = pers.tile([P, NCH * 8], f32, name="affine_t")
            nc.scalar.dma_start(out=affine, in_=affine_d)

        def tap(nm, tiles):
            if not debug:
                return
            cols = tiles[0].shape[-1]
            d = mkdbg(nm, [len(tiles) * P, cols])
            for i, t in enumerate(tiles):
                rows = t.shape[0]
                nc.gpsimd.dma_start(out=d[i * P:i * P + rows, :], in_=t)

        rpool = ctx.enter_context(tc.tile_pool(name="rpool", bufs=1))

        def mktiles(nm, cols=512, dt=f16, n=NCH, tagp=None):
            tagp = tagp or nm
            return [rpool.tile([P, cols], dt, name=f"{nm}{i}", tag=f"{tagp}{i}",
                               bufs=1) for i in range(n)]

        smallp = ctx.enter_context(tc.tile_pool(name="smallp", bufs=1))
        trp = ctx.enter_context(tc.tile_pool(name="trp", bufs=1))
        psA = ctx.enter_context(tc.tile_pool(name="psA", bufs=2, space="PSUM"))
        psB = ctx.enter_context(tc.tile_pool(name="psB", bufs=2, space="PSUM"))

        wt = {}

        def load_wgroup(pool, nm, dram_ap, names, q=None):
            """One contiguous DMA for a group of [P, NCH, D] weights."""
            t = pool.tile([P, NCH, len(names) * D], fp8, name=nm, tag=nm,
                          bufs=1)
            (q or nc.sync).dma_start(
                out=t, in_=dram_ap.rearrange("p (i n) -> p i n", i=NCH))
            for k, n in enumerate(names):
                wt[n] = t[:, :, k * D:(k + 1) * D]
            return t

        def load_act(pool, nm, dram_ap, cols, q=None):
            t = pool.tile([P, NCH, cols], fp8, name=nm, tag=nm, bufs=1)
            (q or nc.scalar).dma_start(
                out=t, in_=dram_ap.rearrange("p (i n) -> p i n", i=NCH))
            return t

        # ------------------------------------------------------------------
        # projections (fp8 DoubleRow over contraction pairs)
        # ------------------------------------------------------------------
        def proj_fm_gen(wn, rhs, ncols, evict, rhs_cs=0):
            """Weight-stationary: out feature-major.  rhs: [P, NCH, *] fp8."""
            ntt = (ncols + 511) // 512
            for j in range(NCH):
                for t in range(ntt):
                    cs = t * 512
                    ce = min(ncols, cs + 512)
                    ps = psA.tile([P, ce - cs], f32, name="proj_ps", tag="pps")
                    for g in range(2):
                        nc.tensor.matmul(
                            ps, wt[wn][:, 2 * g:2 * g + 2, j * P:(j + 1) * P],
                            rhs[:, 2 * g:2 * g + 2, rhs_cs + cs:rhs_cs + ce],
                            start=(g == 0), stop=(g == 1), perf_mode=DR)
                    evict(j, cs, ce, ps)
                    yield

        def drain(gen):
            for _ in gen:
                pass

        def keepwarm(n):
            """Dependency-free dummy matmuls (64-col, ~80ns each) to hold
            the HAM clock gate at K=8/8 through engine-serial chains (PE
            idle > ~3.4us drops the PE clock to 1.2 GHz and the re-warm
            costs another ~3.4us)."""
            dps = psA.tile([1, 64], f32, name="warm_ps", tag="pps")
            for _ in range(n):
                nc.tensor.matmul(dps, ones_f, warm2, start=True,
                                 stop=True, skip_group_check=True)

        def proj_fm(*a, **k):
            drain(proj_fm_gen(*a, **k))

        def proj_v_gen(enc, wn, nkv, vp_list, vpool, ktag):
            """V projection token-major into paired tiles [P,2,H*VH].
            All kv tiles are full 128 rows (host zero-pads)."""
            nkt = nkv // P
            for m in range(nkt):
                ps = psA.tile([P, D], f32, name="v_ps", tag="pps")
                for g in range(2):
                    nc.tensor.matmul(
                        ps, enc[:, 2 * g:2 * g + 2, m * P:(m + 1) * P],
                        wt[wn][:, 2 * g:2 * g + 2, :],
                        start=(g == 0), stop=(g == 1), perf_mode=DR)
                if m % 2 == 0:
                    vt = vpool.tile([P, 2, H * VH], fp8, name=f"{ktag}_v{m//2}",
                                    tag=f"{ktag}_v{m//2}", bufs=1)
                    vp_list.append(vt)
                vt = vp_list[m // 2]
                dst3 = vt[:, m % 2, :].rearrange("p (g c) -> p g c", c=VH)
                src3 = ps.rearrange("p (g c) -> p g c", c=DH)
                nc.vector.tensor_scalar_mul(dst3[:, :, 0:DH], src3, 1.0 / WS)
                nc.gpsimd.memset(dst3[:, :, DH:DH + 1], 1.0)
                yield
            if nkt % 2 == 1:  # odd tile count: zero the unused pair slot
                nc.gpsimd.memset(vp_list[-1][:, 1, :], 0.0)

        def proj_v(*a, **k):
            drain(proj_v_gen(*a, **k))

        # ------------------------------------------------------------------
        # attention
        # ------------------------------------------------------------------
        def attention_gen(qt, kt, vp_list, nkv, at, bias_a, bias_d, causal,
                          ktag, filler=None):
            """qt: [4][P,512] fp8 (2 heads per chunk); kt: [4][P,nkv] fp8
            (A8-scaled, scores psum = A8*logit); vp_list: paired V tiles;
            at: [P,NCH,512] fp8 out (normalized).  exp runs on DVE
            (Schraudolph int8 write) for full tiles / even head-pairs and
            on ACT for causal-diagonal tiles / odd head-pairs."""
            nkt = nkv // P
            npair = (nkt + 1) // 2
            for hp in (0, 2, 1, 3):
                po = [psB.tile([DH + 1, 512], f32, name=f"{ktag}_po{s}",
                               tag="x_po") for s in range(2)]
                for m in range(npair):
                    two = min(2, nkt - 2 * m)
                    pt = trp.tile([P, 2, 1024], fp8, name=f"{ktag}_pt",
                                  tag=f"pt_{ktag}",
                                  bufs={"sa": 3, "ck": 1}.get(ktag, 2))
                    for ti in range(two):
                        kt_i = 2 * m + ti
                        ps2 = psB.tile([P, 1024], f32, name=f"{ktag}_ps",
                                       tag="x_ps")
                        d = kt_i - (nkt - 4) if causal else -1
                        lo = d * P if (causal and d > 0) else 0
                        for s in range(2):
                            nc.tensor.matmul(
                                ps2[:, s * 512 + lo:(s + 1) * 512],
                                kt[hp][s * DH:(s + 1) * DH,
                                       kt_i * P:(kt_i + 1) * P],
                                qt[hp][s * DH:(s + 1) * DH, lo:512],
                                start=True, stop=True)
                        if causal and d >= 0:
                            for s in range(2):
                                o = s * 512
                                nc.vector.tensor_add(
                                    ps2[:, o + d * P:o + (d + 1) * P],
                                    ps2[:, o + d * P:o + (d + 1) * P], stair)
                                if d > 0:
                                    nc.gpsimd.memset(pt[:, ti, o:o + d * P], 0.0)
                                nc.scalar.activation(
                                    pt[:, ti, o + d * P:o + 512],
                                    ps2[:, o + d * P:o + 512],
                                    AF.Exp, bias=neg2[:, :],
                                    scale=1.0 / A8)
                        elif (kt_i + hp // 2) % 2 == 0:
                            bias = B0
                            if (not causal) and bias_d is not None \
                                    and kt_i == nkt - 1:
                                bias = bias_d
                            nc.vector.tensor_scalar(
                                pt[:, ti, :].bitcast(i8), ps2, bias, 0.0,
                                op0=OP.add, op1=OP.max)
                        else:
                            bias = neg2[:, :]
                            if bias_a is not None and kt_i == nkt - 1:
                                bias = bias_a
                            nc.scalar.activation(pt[:, ti, :], ps2, AF.Exp,
                                                 bias=bias, scale=1.0 / A8)
                        if two == 1:
                            nc.gpsimd.memset(pt[:, 1, :], 0.0)
                        if filler is not None:
                            if next(filler, False) is False:
                                keepwarm(4)
                        else:
                            keepwarm(4)
                    for s in range(2):
                        nc.tensor.matmul(
                            po[s],
                            vp_list[m][:, :, (2 * hp + s) * VH:
                                       (2 * hp + s) * VH + DH + 1],
                            pt[:, :, s * 512:(s + 1) * 512],
                            start=(m == 0), stop=(m == npair - 1),
                            perf_mode=DR)
                # denominator rows (ones-col in V): stage through SBUF
                # (custom-DVE bit tricks misread PSUM), recip on DVE, f16
                # cast on ACT, broadcast into nb's 64-row halves via two
                # small PE matmuls; numerators leave PSUM through ACT
                # copies (QS folded in), then one DVE mul applies 1/n
                nb = psA.tile([P, 512], f32, name=f"{ktag}_nb", tag="pps")
                at_tmp = trp.tile([P, 512], f16, name=f"{ktag}_att",
                                  tag="at_tmp", bufs=3)
                for s in range(2):
                    den_sb = smallp.tile([1, 512], f32,
                                         name=f"{ktag}_den{s}",
                                         tag="den_sb", bufs=4)
                    nc.scalar.activation(den_sb, po[s][DH:DH + 1, :],
                                         AF.Copy)
                    ninv = smallp.tile([1, 512], f32,
                                       name=f"{ktag}_ninv{s}",
                                       tag="ninv", bufs=4)
                    nc.vector.reciprocal_approx_fast(ninv, den_sb)
                    ninv16 = smallp.tile([1, 512], f16,
                                         name=f"{ktag}_ninv16{s}",
                                         tag="ninv16", bufs=4)
                    nc.scalar.activation(ninv16, ninv, AF.Copy)
                    nc.tensor.matmul(nb[s * DH:(s + 1) * DH, :],
                                     ones_row[:, 0:DH], ninv16,
                                     start=True, stop=True)
                    nc.scalar.activation(at_tmp[s * DH:(s + 1) * DH, :],
                                         po[s][0:DH, :], AF.Copy, scale=QS)
                nc.vector.tensor_mul(at[:, hp, :], at_tmp, nb)
                yield

        def attention(*a, **k):
            drain(attention_gen(*a, **k))

        # ------------------------------------------------------------------
        # layernorm (f16 residual in, f16 + fp8 out)
        # ------------------------------------------------------------------
        def layernorm(r_tiles, out16, out8, ln_idx, warm=(40, 15)):
            sq = [trp.tile([P, 512], f16, name=f"ln{ln_idx}_sq", tag="ln_sq",
                           bufs=2) for _ in range(NCH)]
            for j in range(NCH):
                if j < 2:
                    nc.scalar.activation(sq[j], r_tiles[j], AF.Square)
                elif j == 2:
                    nc.vector.tensor_mul(sq[j], r_tiles[j], r_tiles[j])
                else:
                    nc.gpsimd.tensor_mul(sq[j], r_tiles[j], r_tiles[j])
            ps_s = psB.tile([1, 512], f32, name="ln_ps_s", tag="x_po")
            ps_q = psB.tile([1, 512], f32, name="ln_ps_q", tag="x_po")
            for j in range(NCH):
                nc.tensor.matmul(ps_s, ones_f, r_tiles[j],
                                 start=(j == 0), stop=(j == NCH - 1))
            for j in range(NCH):
                nc.tensor.matmul(ps_q, ones_f, sq[j],
                                 start=(j == 0), stop=(j == NCH - 1))
            keepwarm(warm[0])
            mean16 = smallp.tile([1, 512], f16, name="ln_mean16",
                                 tag="ln_stat", bufs=3)
            nc.scalar.activation(mean16, ps_s, AF.Copy, scale=1.0 / D)
            msq = smallp.tile([1, 512], f32, name="ln_msq",
                              tag="ln_stat", bufs=3)
            nc.scalar.activation(msq, ps_s, AF.Square, scale=1.0 / D)
            var = smallp.tile([1, 512], f32, name="ln_var",
                              tag="ln_stat", bufs=3)
            nc.vector.scalar_tensor_tensor(var, ps_q, 1.0 / D, msq,
                                           op0=OP.mult, op1=OP.subtract)
            # rstd via bitcast fast-rsqrt + one Newton step (no Ln table -
            # mixing Ln into the Exp-heavy kernel forces ~1.3us ACT table
            # swaps around every layernorm)
            lf = smallp.tile([1, 512], f32, name="ln_lf",
                             tag="ln_stat", bufs=3)
            nc.vector.tensor_scalar(lf, var.bitcast(mybir.dt.int32),
                                    1064866805.0, 8.2629582e-08,
                                    op0=OP.subtract, op1=OP.mult)
            rstd0 = smallp.tile([1, 512], f32, name="ln_rstd0",
                                tag="ln_nr", bufs=3)
            nc.scalar.activation(rstd0, lf, AF.Exp, scale=-0.5)
            t1 = smallp.tile([1, 512], f32, name="ln_t1", tag="ln_nr", bufs=3)
            nc.vector.tensor_mul(t1, rstd0, rstd0)
            nc.vector.tensor_mul(t1, t1, var)
            nc.vector.tensor_scalar(t1, t1, -0.5, 1.5, op0=OP.mult, op1=OP.add)
            rstd = smallp.tile([1, 512], f16, name="ln_rstd",
                               tag="ln_stat", bufs=3)
            nc.vector.tensor_mul(rstd, rstd0, t1)
            meanb = psB.tile([P, 512], f32, name="ln_meanb", tag="x_po")
            nc.tensor.matmul(meanb, ones_row, mean16, start=True, stop=True)
            rstdb = psB.tile([P, 512], f32, name="ln_rstdb", tag="x_po")
            nc.tensor.matmul(rstdb, ones_row, rstd, start=True, stop=True)
            keepwarm(warm[1])
            mb_sb = trp.tile([P, 512], f16, name="ln_mb_sb", tag="ln_mb",
                             bufs=1)
            rb_sb = trp.tile([P, 512], f16, name="ln_rb_sb", tag="ln_rb",
                             bufs=1)
            nc.scalar.activation(mb_sb, meanb, AF.Copy)
            nc.vector.tensor_copy(rb_sb, rstdb)
            for j in range(NCH):
                tmp = trp.tile([P, 512], f16, name="ln_tmp", tag="ln_tmp",
                               bufs=2)
                if j >= 2:
                    nc.gpsimd.tensor_sub(tmp, r_tiles[j], mb_sb)
                    nc.gpsimd.tensor_mul(out16[j], tmp, rb_sb)
                else:
                    nc.vector.tensor_sub(tmp, r_tiles[j], mb_sb)
                    nc.vector.tensor_mul(out16[j], tmp, rb_sb)
                if apply_affine:
                    g = affine[:, ln_idx * 2 * NCH + j:
                               ln_idx * 2 * NCH + j + 1]
                    b = affine[:, ln_idx * 2 * NCH + NCH + j:
                               ln_idx * 2 * NCH + NCH + j + 1]
                    nc.vector.tensor_scalar(out16[j], out16[j], g, b,
                                            op0=OP.mult, op1=OP.add)
                if out8 is not None:
                    if j >= 2:
                        nc.gpsimd.tensor_copy(out8[:, j, :], out16[j])
                    else:
                        nc.vector.tensor_copy(out8[:, j, :], out16[j])

        # ==================================================================
        # emission
        # ==================================================================
        r1 = mktiles("r1", tagp="rA")
        y16 = mktiles("y", tagp="lnA")
        r2 = mktiles("r2", tagp="rB")
        z16 = mktiles("z", tagp="lnB")

        ctsb = ctx.enter_context(tc.tile_pool(name="tail_sb", bufs=1))
        ccsb_cm = tc.tile_pool(name="cc_sb", bufs=1)
        ccsb = ccsb_cm.__enter__()
        sasb_cm = tc.tile_pool(name="sa_sb", bufs=1)
        sasb = sasb_cm.__enter__()

        # --- stage 1: self attention ---
        # sync ring: saw -> xin -> xq16 -> w1 -> w2 (FIFO; sa starts ~4us)
        # scalar ring: consts (above) -> enc_cc -> ccw -> enc_ct -> ctw
        load_wgroup(sasb, "saw", saw_d, SAW_ORDER, q=nc.sync)
        xkv = load_act(sasb, "xkv", xin_d, KV, q=nc.sync)
        xq16_t = sasb.tile([P, NCH, 512], f16, name="xq16", tag="xq16",
                           bufs=1)
        nc.sync.dma_start(out=xq16_t,
                          in_=xq16_d.rearrange("p (i n) -> p i n", i=NCH))
        xq16 = [xq16_t[:, i, :] for i in range(NCH)]
        enc_cc_t = load_act(ccsb, "enc_cc", enc_cc_d, (kts_cc + 1) * P)
        srcl = enc_cc_t[:, :, 0:kts_cc * P]
        kwe = enc_cc_t[:, :, kts_cc * P:(kts_cc + 1) * P]
        load_wgroup(ccsb, "ccw", ccw_d, CCW_ORDER, q=nc.scalar)
        tmpl = load_act(ctsb, "enc_ct", enc_ct_d, kts_ct * P)
        load_wgroup(ctsb, "ctw", ctw_d, CTW_ORDER, q=nc.scalar)
        qt = [sasb.tile([P, 512], fp8, name=f"sa_q{i}", tag=f"sa_q{i}",
                        bufs=1) for i in range(NCH)]
        ktl = [sasb.tile([P, KV], fp8, name=f"sa_k{i}", tag=f"sa_k{i}",
                         bufs=1) for i in range(NCH)]
        keepwarm(80)

        def evict_scale(dst, scale):
            def ev(j, cs, ce, ps):
                if j % 2 == 0:
                    nc.vector.tensor_scalar_mul(dst[j][:, cs:ce], ps, scale)
                else:
                    nc.scalar.activation(dst[j][:, cs:ce], ps, AF.Copy,
                                         scale=scale)
            return ev

        proj_fm("sa_wk", xkv, KV, evict_scale(ktl, A8 / 2048.0))
        vts = []
        proj_v(xkv, "sa_wv", KV, vts, sasb, "sa")
        proj_fm("sa_wq", xkv, 512, evict_scale(qt, QS / WS), rhs_cs=QOFF)
        at = sasb.tile([P, NCH, 512], fp8, name="sa_at", tag="sa_at", bufs=1)
        # hoist: all cross-attn K/V projections as FILLER inside sa
        # attention so the PE queue stays dense through the exp stalls.
        cc_kt = [ccsb.tile([P, kts_cc * P], fp8, name=f"cc_k{i}",
                           tag=f"cc_k{i}", bufs=1) for i in range(NCH)]
        cc_vts = []
        ck_kt = [ccsb.tile([P, KWP], fp8, name=f"ck_k{i}", tag=f"ck_k{i}",
                           bufs=1) for i in range(NCH)]
        ck_vts = []
        ct_kt = [ctsb.tile([P, kts_ct * P], fp8, name=f"ct_k{i}",
                           tag=f"ct_k{i}", bufs=1) for i in range(NCH)]
        ct_vts = []

        def chain(*gens):
            for g in gens:
                yield from g

        # cc+ck K/V fill the sa-attention exp stalls; ct K/V is held back
        # so real PE + eviction work remains to fill the LN1 serial chain
        kv_filler = chain(
            proj_fm_gen("cc_wk", srcl, kts_cc * P,
                        evict_scale(cc_kt, A8 / 2048.0)),
            proj_v_gen(srcl, "cc_wv", kts_cc * P, cc_vts, ccsb, "cc"),
            proj_fm_gen("ck_wk", kwe, KWP, evict_scale(ck_kt, A8 / 2048.0)),
            proj_v_gen(kwe, "ck_wv", KWP, ck_vts, ccsb, "ck"),
        )
        late_kv = chain(
            proj_fm_gen("ct_wk", tmpl, kts_ct * P,
                        evict_scale(ct_kt, A8 / 2048.0)),
            proj_v_gen(tmpl, "ct_wv", kts_ct * P, ct_vts, ctsb, "ct"),
        )
        attention(qt, ktl, vts, KV, at, None, None, True, "sa",
                  filler=kv_filler)

        def evict_resid(dst, src16, scale):
            def ev(j, cs, ce, ps):
                nc.vector.scalar_tensor_tensor(
                    dst[j][:, cs:ce], ps, scale, src16[j][:, cs:ce],
                    op0=OP.mult, op1=OP.add)
            return ev

        keepwarm(20)
        proj_fm("sa_wo", at, 512, evict_resid(r1, xq16, 1.0 / (QS * WS)))
        tap("r1", r1)
        y8 = ccsb.tile([P, NCH, 512], fp8, name="y8", tag="y8", bufs=1)
        # ct K/V projections (late_kv) provide real PE work through LN1's
        # serial chain, so LN1 needs almost no warm filler
        layernorm(r1, y16, y8, 0, warm=(8, 4))
        drain(kv_filler)
        drain(late_kv)
        tap("y", y16)
        sasb_cm.__exit__(None, None, None)

        # --- stage 2: cc + ck cross attention + gate ---
        cc_qt = [ccsb.tile([P, 512], fp8, name=f"cc_q{i}", tag=f"cc_q{i}",
                           bufs=1) for i in range(NCH)]
        proj_fm("cc_wq", y8, 512, evict_scale(cc_qt, QS / WS))
        cc_at = ccsb.tile([P, NCH, 512], fp8, name="cc_at", tag="cc_at",
                          bufs=1)
        ck_qt = [ccsb.tile([P, 512], fp8, name=f"ck_q{i}", tag=f"ck_q{i}",
                           bufs=1) for i in range(NCH)]
        cc_filler = proj_fm_gen("ck_wq", y8, 512, evict_scale(ck_qt, QS / WS))
        ck_at = ccsb.tile([P, NCH, 512], fp8, name="ck_at", tag="ck_at",
                          bufs=1)
        attention(cc_qt, cc_kt, cc_vts, kts_cc * P, cc_at,
                  ccbiasA, ccbiasD, False, "cc", filler=cc_filler)
        drain(cc_filler)
        attention(ck_qt, ck_kt, ck_vts, KWP, ck_at, kwbiasA, kwbiasD,
                  False, "ck")
        y2c = mktiles("y2c", tagp="y2c")
        y2k = mktiles("y2k", tagp="y2k")
        keepwarm(20)
        proj_fm("cc_wo", cc_at, 512, evict_scale(y2c, 1.0 / (QS * WS)))
        proj_fm("ck_wo", ck_at, 512, evict_scale(y2k, 1.0 / (QS * WS)))

        # --- gate ---
        ps_g = psB.tile([1, 512], f32, name="gate_ps", tag="x_po")
        for i in range(NCH):
            nc.tensor.matmul(ps_g, gwa_t[:, i:i + 1], y2c[i],
                             start=(i == 0), stop=False)
        for i in range(NCH):
            nc.tensor.matmul(ps_g, gwb_t[:, i:i + 1], y2k[i],
                             start=False, stop=(i == NCH - 1))
        ge = smallp.tile([1, 512], f32, name="gate_e", tag="gate_e", bufs=2)
        nc.scalar.activation(ge, ps_g, AF.Exp, scale=-1.0, bias=gb_t[:, :])
        ge1 = smallp.tile([1, 512], f32, name="gate_e1", tag="gate_e", bufs=2)
        nc.vector.tensor_scalar_add(ge1, ge, 1.0)
        gg = smallp.tile([1, 512], f32, name="gate_g", tag="gate_g", bufs=2)
        nc.vector.reciprocal_approx_fast(gg, ge1)
        gg16 = smallp.tile([1, 512], f16, name="gate_g16", tag="gate_g16",
                           bufs=1)
        nc.vector.tensor_copy(gg16, gg)
        keepwarm(35)
        ggb = psB.tile([P, 512], f32, name="gate_gb", tag="x_po")
        nc.tensor.matmul(ggb, ones_row, gg16, start=True, stop=True)
        ggb_sb = trp.tile([P, 512], f16, name="ggb_sb", tag="ggb_sb", bufs=1)
        nc.scalar.activation(ggb_sb, ggb, AF.Copy)
        # r2 = y + y2k + g*(y2c - y2k)   (blends split DVE / gpsimd)
        for j in range(NCH):
            dt_ = trp.tile([P, 512], f16, name="gate_dt", tag="gate_dt",
                           bufs=4)
            if j >= 2:
                nc.gpsimd.tensor_sub(dt_, y2c[j], y2k[j])
                nc.gpsimd.tensor_mul(dt_, dt_, ggb_sb)
                nc.gpsimd.tensor_add(r2[j], y16[j], y2k[j])
                nc.gpsimd.tensor_add(r2[j], r2[j], dt_)
            else:
                nc.vector.tensor_sub(dt_, y2c[j], y2k[j])
                nc.vector.tensor_mul(dt_, dt_, ggb_sb)
                nc.vector.tensor_add(r2[j], y16[j], y2k[j])
                nc.vector.tensor_add(r2[j], r2[j], dt_)
        tap("r2", r2)
        ccsb_cm.__exit__(None, None, None)
        ffsb = ctx.enter_context(tc.tile_pool(name="ff_sb", bufs=1))
        w1t = ffsb.tile([P, NCH, DFF], f16, name="w1_t", tag="w1_t")
        nc.sync.dma_start(out=w1t,
                          in_=w1d.rearrange("p (i n) -> p i n", i=NCH))
        w2t = ffsb.tile([P, DFF // P, D], fp8, name="w2_t", tag="w2_t")
        nc.sync.dma_start(out=w2t,
                          in_=w2d.rearrange("p (i n) -> p i n", i=DFF // P))
        z8 = ctsb.tile([P, NCH, 512], fp8, name="z8", tag="z8", bufs=1)
        layernorm(r2, z16, z8, 1)
        tap("z", z16)

        # --- stage 3: ct cross attention ---
        r3 = mktiles("r3", tagp="rA")
        ze16 = mktiles("ze", tagp="lnA")
        ct_qt = [ctsb.tile([P, 512], fp8, name=f"ct_q{i}", tag=f"ct_q{i}",
                           bufs=1) for i in range(NCH)]
        proj_fm("ct_wq", z8, 512, evict_scale(ct_qt, QS / WS))
        ct_at = ctsb.tile([P, NCH, 512], fp8, name="ct_at", tag="ct_at",
                          bufs=1)
        attention(ct_qt, ct_kt, ct_vts, kts_ct * P, ct_at,
                  ctbiasA, ctbiasD, False, "ct")
        keepwarm(20)
        proj_fm("ct_wo", ct_at, 512, evict_resid(r3, z16, 1.0 / (QS * WS)))
        tap("r3", r3)
        layernorm(r3, ze16, None, 2)
        tap("ze", ze16)

        # --- stage 4: FFN ---
        ht = ffsb.tile([P, DFF // P, 512], fp8, name="ff_h", tag="ff_h",
                       bufs=1)
        for jf in range(DFF // P):
            ps = psA.tile([P, 512], f32, name="ff_ps", tag="pps")
            for i in range(NCH):
                nc.tensor.matmul(ps, w1t[:, i, jf * P:(jf + 1) * P],
                                 ze16[i], start=(i == 0), stop=(i == NCH - 1))
            if jf % 2 == 0:
                nc.scalar.activation(ht[:, jf, :], ps, AF.Relu, scale=QS)
            else:
                nc.vector.tensor_scalar(ht[:, jf, :], ps, QS, 0.0,
                                        op0=OP.mult, op1=OP.max)
        r4 = mktiles("r4", tagp="rB")
        for j in range(NCH):
            ps = psA.tile([P, 512], f32, name="ff_ps2", tag="pps")
            for g in range(DFF // P // 2):
                nc.tensor.matmul(ps, w2t[:, 2 * g:2 * g + 2, j * P:(j + 1) * P],
                                 ht[:, 2 * g:2 * g + 2, :], start=(g == 0),
                                 stop=(g == DFF // P // 2 - 1), perf_mode=DR)
            nc.vector.scalar_tensor_tensor(r4[j], ps, 1.0 / (QS * WS),
                                           ze16[j], op0=OP.mult, op1=OP.add)
        fin = [trp.tile([P, 512], f16, name=f"fin{i}", tag=f"fin{i}",
                        bufs=1) for i in range(NCH)]
        layernorm(r4, fin, None, 3)
        outq = [nc.sync, nc.scalar, nc.sync, nc.scalar]
        for j in range(NCH):
            outq[j].dma_start(out=outT[j * P:(j + 1) * P, :], in_=fin[j])

    nc.compile()
    return nc


# ---------------------------------------------------------------------------
# host-side input preparation
# ---------------------------------------------------------------------------

def _fp8(x, scale=1.0):
    return np.ascontiguousarray(
        (np.asarray(x, F32) * scale).astype(FP8))


def _tile_feat(a):
    """[D, N] -> [P, NCH, N]; element (p, i, n) = a[i*128 + p, n]."""
    a = np.asarray(a)
    return a.reshape(NCH, P, a.shape[1]).transpose(1, 0, 2)


def _prep_shared(inputs):
    sh = {}
    w8 = {f"{n}_{p}": _fp8(inputs[f"{n}_{p}"], WS)
          for n in ("sa", "cc", "ct", "ck") for p in ("wq", "wk", "wv", "wo")}

    def pack(names):
        return np.ascontiguousarray(np.concatenate(
            [_tile_feat(w8[n]) for n in names], axis=2)).reshape(P, -1)

    sh["saw"] = pack(SAW_ORDER)
    sh["ccw"] = pack(CCW_ORDER)
    sh["ctw"] = pack(CTW_ORDER)
    sh["ffn_w1"] = np.ascontiguousarray(
        _tile_feat(np.asarray(inputs["ffn_w1"], F32).astype(F16))
    ).reshape(P, -1)
    w2 = _fp8(inputs["ffn_w2"], WS)   # [DFF, D]
    sh["ffn_w2"] = np.ascontiguousarray(
        w2.reshape(DFF // P, P, D).transpose(1, 0, 2)).reshape(P, -1)
    gw = np.asarray(inputs["gate_w"], F32)
    sh["gw"] = np.ascontiguousarray(np.concatenate(
        [gw[:D, 0].reshape(NCH, P).T, gw[D:, 0].reshape(NCH, P).T],
        axis=1).astype(F16))
    kl, ql = np.arange(P)[:, None], np.arange(P)[None, :]
    sh["stair"] = np.where(kl <= ql, 0.0, NEG).astype(F16)
    return sh


def _len_bias(L, kts, base_val, width=P):
    base = (kts - 1) * P
    idx = base + np.arange(width)
    return (base_val + np.where(idx < L, 0.0, NEG)).astype(F32)[:, None]


def _prep_core(inputs, sh, b, qh, kts_cc, kts_ct):
    KVn = 512 * (qh + 1)
    m = dict(sh)
    xT = np.asarray(inputs["x"][b], F32).T  # [D, T]
    m["xin"] = np.ascontiguousarray(
        _tile_feat(_fp8(xT[:, :KVn]))).reshape(P, -1)
    m["xq16"] = np.ascontiguousarray(
        _tile_feat(xT[:, qh * 512:(qh + 1) * 512].astype(F16))).reshape(P, -1)
    Ls = int(inputs["source_code_len"][b])
    st = np.zeros((D, kts_cc * P), FP8)
    st[:, :Ls] = _fp8(np.asarray(inputs["source_code_enc"][b], F32)[:Ls].T)
    kwp = np.zeros((D, KWP), FP8)
    kwp[:, :KW] = _fp8(np.asarray(inputs["keywords_enc"][b], F32).T)
    m["enc_cc"] = np.ascontiguousarray(np.concatenate(
        [_tile_feat(st), _tile_feat(kwp)], axis=2)).reshape(P, -1)
    Lt = int(inputs["template_len"][b])
    tt = np.zeros((D, kts_ct * P), FP8)
    tt[:, :Lt] = _fp8(np.asarray(inputs["template_enc"][b], F32)[:Lt].T)
    m["enc_ct"] = np.ascontiguousarray(_tile_feat(tt)).reshape(P, -1)
    Lk = int(inputs["keywords_len"][b])
    bz = np.zeros((P, 8), F32)
    bz[:, 0:1] = _len_bias(Ls, kts_cc, -2.0)
    bz[:, 1:2] = _len_bias(Ls, kts_cc, B0)
    bz[:, 2:3] = _len_bias(Lt, kts_ct, -2.0)
    bz[:, 3:4] = _len_bias(Lt, kts_ct, B0)
    bz[:, 4:5] = _len_bias(Lk, 1, -2.0, P)
    bz[:, 5:6] = _len_bias(Lk, 1, B0, P)
    m["biases"] = bz
    return m


# ---------------------------------------------------------------------------
# concurrent multi-program PJRT runner
# ---------------------------------------------------------------------------

def _run_groups(groups):
    import jax
    import numpy as _np
    from jax.sharding import Mesh, PartitionSpec
    from jax.experimental.shard_map import shard_map
    from concourse import bass2jax
    from concourse.bass2jax import (_bass_exec_p, install_neuronx_cc_hook,
                                    partition_id_tensor)

    install_neuronx_cc_hook()
    devices = jax.devices()

    def make_launch(nc, core_ids, in_maps):
        pname = (nc.partition_id_tensor.name
                 if nc.partition_id_tensor else None)
        in_names, out_names, out_avals, zero_outs = [], [], [], []
        for alloc in nc.m.functions[0].allocations:
            if not isinstance(alloc, mybir.MemoryLocationSet):
                continue
            name = alloc.memorylocations[0].name
            if alloc.kind == "ExternalInput":
                if name == pname:
                    continue
                in_names.append(name)
            elif alloc.kind == "ExternalOutput":
                shape = tuple(alloc.tensor_shape)
                dtype = mybir.dt.np(alloc.dtype)
                out_names.append(name)
                out_avals.append(jax.core.ShapedArray(shape, dtype))
                zero_outs.append(_np.zeros(shape, dtype))
        n_params, n_outs = len(in_names), len(out_avals)
        all_in_names = in_names + out_names
        if pname is not None:
            all_in_names = all_in_names + [pname]

        def _body(*args):
            operands = list(args)
            if pname is not None:
                operands.append(partition_id_tensor())
            outs = _bass_exec_p.bind(
                *operands, out_avals=tuple(out_avals),
                in_names=tuple(all_in_names), out_names=tuple(out_names),
                lowering_input_output_aliases=(),
                sim_require_finite=False, sim_require_nnan=False, nc=nc)
            return tuple(outs)

        donate = tuple(range(n_params, n_params + n_outs))
        devs = [devices[c] for c in core_ids]
        if len(core_ids) == 1:
            fn = jax.jit(_body, donate_argnums=donate, keep_unused=True,
                         device=devs[0])
            args = [in_maps[0][nm] for nm in in_names] + list(zero_outs)
            out_arrs = fn(*args)
            return out_names, out_avals, out_arrs, None
        mesh = Mesh(_np.asarray(devs), ("core",))
        in_specs = (PartitionSpec("core"),) * (n_params + n_outs)
        out_specs = (PartitionSpec("core"),) * n_outs
        fn = jax.jit(shard_map(_body, mesh=mesh, in_specs=in_specs,
                               out_specs=out_specs, check_rep=False),
                     donate_argnums=donate, keep_unused=True)
        cat = [_np.concatenate([_np.asarray(m[nm]) for m in in_maps], axis=0)
               for nm in in_names]
        catz = [_np.zeros((len(core_ids) * z.shape[0], *z.shape[1:]), z.dtype)
                for z in zero_outs]
        out_arrs = fn(*cat, *catz)
        return out_names, out_avals, out_arrs, len(core_ids)

    last_err = None
    for _attempt in range(3):
        try:
            launched = []
            for nc, core_ids, in_maps in groups:
                launched.append((core_ids, make_launch(nc, core_ids, in_maps)))
            results = {}
            for core_ids, (out_names, out_avals, out_arrs, ncores) in launched:
                if ncores is None:
                    results[core_ids[0]] = {nm: _np.asarray(out_arrs[i])
                                            for i, nm in enumerate(out_names)}
                else:
                    for ci, c in enumerate(core_ids):
                        results[c] = {
                            nm: _np.asarray(out_arrs[i]).reshape(
                                ncores, *out_avals[i].shape)[ci]
                            for i, nm in enumerate(out_names)}
            return results
        except Exception as e:
            last_err = e
            import time as _time
            _time.sleep(2.0)
    raise last_err


_PROGRAM_CACHE = {}
_CACHE_LOCK = threading.Lock()


def _get_program(key):
    with _CACHE_LOCK:
        if key in _PROGRAM_CACHE:
            return _PROGRAM_CACHE[key]
    qh, kts_cc, kts_ct, gate_b, aff = key
    nc = build_program(qh, kts_cc, kts_ct, gate_b=gate_b, apply_affine=aff)
    with _CACHE_LOCK:
        _PROGRAM_CACHE[key] = nc
    return nc


# ---------------------------------------------------------------------------
# entry point
# ---------------------------------------------------------------------------

def kernel(**inputs):
    inputs = {k: np.asarray(v) for k, v in inputs.items()}
    gate_b = float(inputs["gate_b"].reshape(-1)[0])
    aff = not all(
        np.all(inputs[f"ln{j}_g"] == 1.0) and np.all(inputs[f"ln{j}_b"] == 0.0)
        for j in range(1, 5))
    affine_arr = None
    if aff:
        affine_arr = np.zeros((P, NCH * 8), F32)
        for ln in range(4):
            g = inputs[f"ln{ln + 1}_g"].astype(F32).reshape(NCH, P).T
            bb = inputs[f"ln{ln + 1}_b"].astype(F32).reshape(NCH, P).T
            affine_arr[:, ln * 2 * NCH: ln * 2 * NCH + NCH] = g
            affine_arr[:, ln * 2 * NCH + NCH: (ln + 1) * 2 * NCH] = bb

    sh = _prep_shared(inputs)
    core_keys, core_maps = [], []
    for c in range(8):
        b, qh = c // 2, c % 2
        kts_cc = max(1, -(-int(inputs["source_code_len"][b]) // P))
        kts_ct = max(1, -(-int(inputs["template_len"][b]) // P))
        key = (qh, kts_cc, kts_ct, gate_b, aff)
        m = _prep_core(inputs, sh, b, qh, kts_cc, kts_ct)
        if aff:
            m["ln_affine"] = affine_arr
        core_keys.append(key)
        core_maps.append(m)

    distinct = sorted(set(core_keys))
    threads = [threading.Thread(target=_get_program, args=(k,))
               for k in distinct]
    for t in threads:
        t.start()
    for t in threads:
        t.join()

    groups = []
    for key in distinct:
        cores = [c for c in range(8) if core_keys[c] == key]
        groups.append((_get_program(key), cores, [core_maps[c] for c in cores]))

    results = _run_groups(groups)

    out = np.empty((B, T, D), np.float32)
    for c in range(8):
        b, qh = c // 2, c % 2
        out[b, qh * 512:(qh + 1) * 512, :] = \
            results[c]["outT"].astype(np.float32).T
    return out

